# revision 15
# baseline (speedup 1.0000x reference)
"""CapsuleLayer (dynamic routing) Trainium2 Bass kernel.

Full inputs:  x [128, 512, 256] f32, W [32, 512, 16, 256] f32
Full output:  [128, 32, 16] f32

Sharding: split the input-capsule dim N=512 across 8 cores (64 each).
Each core computes its slice of inputs_hat = einsum('bni,mndi->bmnd') in
bf16 (inputs pre-converted on host; halves the HBM traffic to ~20MB/core,
the roofline), keeps it SBUF-resident as [b=128 part, (n_loc, d, m) free]
bf16, and runs the 3 routing iterations locally (softmax over m is fully
local); the per-core partial s = sum_n c*inputs_hat is AllReduced (128KB
bf16) once per iteration.

Routing avoids DVE tensor_reduce (always 1 elem/cycle) in favor of
pairwise in-place add-trees on TensorTensor ops, which run at 2x in bf16
with packed innermost access.  Work is split by n-rows between DVE and
GPSIMD, each engine cascading its own rows through mul + tree so there
are no cross-engine deps until the softmax.  Iteration i's logits are
recomputed from o_sum = sum_t o_t (b-updates are linear in o), so no
logit state is carried.  Iteration 1 (uniform c) reduces to
s1 = sum_n ih / M, accumulated with bf16 adds hidden under the einsum
DMA.  Squash uses exp(-0.5*ln(u+eps)) for rsqrt so the whole kernel
lives in one ACT function set (no table reloads).
"""

import sys

sys.path.insert(0, "/opt/trn_rl_repo")

import numpy as np

import concourse.bacc as bacc
import concourse.mybir as mybir
import concourse.tile as tile
from concourse.bass_utils import run_bass_kernel_spmd

N_CORES = 8
B, N, I = 128, 512, 256
M, D = 32, 16
DM = D * M                 # ih free layout is (d, m): m innermost
NL = N // N_CORES          # 64 local input capsules per core
NH = NL // 2               # routing processes n in two halves of 32
EPS = 1e-7
F32 = mybir.dt.float32
BF16 = mybir.dt.bfloat16

NB = 8                     # n-block size per xt DMA / wt DMA pair

# debug/profiling knobs (defaults = full kernel)
# gn: GPSIMD n-rows per 32-row half (DVE gets 32-gn)
_cfg = {"routing": True, "iters": (2, 3), "reps": 1, "no_cc": False, "gn": 6}

LN = mybir.ActivationFunctionType.Ln
EXP = mybir.ActivationFunctionType.Exp


class _SteerActTables:
    """Compile-scoped: hide exp/ln from every act-func set except
    natural_log_exp_and_others so the table selector settles on the one
    set that covers all our activations (Copy/Exp/Ln) — a single table
    load for the whole kernel instead of per-iteration thrash.  Set ids
    keep their act_info.json indices, so the chosen table is still valid;
    the original resolver is restored on exit."""

    def __enter__(self):
        self._orig = bacc.get_activation_tables

        def steered(arch):
            tabs = self._orig(arch)
            keep = "natural_log_exp_and_others"
            out = {}
            for name, funcs in tabs.items():
                if name != keep:
                    funcs = funcs - {mybir.ActivationFunctionType.Exp,
                                     mybir.ActivationFunctionType.Ln}
                out[name] = funcs
            return out

        bacc.get_activation_tables = steered
        return self

    def __exit__(self, *a):
        bacc.get_activation_tables = self._orig
        return False


def _rsqrt(nc, pool, u, eps_t, tag):
    """exp(-0.5*ln(u+eps)) on ACT: both funcs in one act set."""
    lg = pool.tile([128, M], F32, tag="lg" + tag)
    nc.scalar.activation(lg, u, LN, bias=eps_t[:, 0:1])
    ri = pool.tile([128, M], F32, tag="ri" + tag)
    nc.scalar.activation(ri, lg, EXP, scale=-0.5)
    return ri


def _squash(tc, pool, s_src, eps_t, tag=""):
    """o = squash(s) over d; s [128, (d, m)] bf16 SBUF. Returns o (bf16)."""
    nc = tc.nc
    sq = pool.tile([128, DM], F32, tag="sq" + tag)
    nc.vector.tensor_mul(sq, s_src, s_src)
    s2 = pool.tile([128, M], F32, tag="s2" + tag)
    nc.vector.tensor_reduce(
        s2, sq.rearrange("p (d m) -> p m d", d=D),
        axis=mybir.AxisListType.X, op=mybir.AluOpType.add,
    )
    ri = _rsqrt(nc, pool, s2, eps_t, tag)   # 1/sqrt(s2+eps)
    one_p = pool.tile([128, M], F32, tag="one_p" + tag)
    nc.vector.tensor_scalar_add(one_p, s2, 1.0)
    rec = pool.tile([128, M], F32, tag="rec" + tag)
    nc.vector.reciprocal(rec, one_p)
    pr = pool.tile([128, M], F32, tag="pr" + tag)
    nc.vector.tensor_mul(pr, s2, rec)
    scl = pool.tile([128, M], F32, tag="scl" + tag)
    nc.vector.tensor_mul(scl, pr, ri)       # scale = s2/(1+s2)/sqrt(s2+eps)
    o = pool.tile([128, DM], BF16, tag="o" + tag)
    nc.vector.tensor_mul(
        o.rearrange("p (d m) -> p d m", d=D),
        s_src.rearrange("p (d m) -> p d m", d=D),
        scl.unsqueeze(1).broadcast_to([128, D, M]),
    )
    return o


def _allreduce(tc, dram_pool, sb_pool, src, idx, n_cores=N_CORES):
    """AllReduce [128, DM] bf16 across the cores. Returns SBUF tile."""
    nc = tc.nc
    bin_ = dram_pool.tile([128, DM], BF16, tag=f"arin{idx}")
    bout = dram_pool.tile([128, DM], BF16, tag=f"arout{idx}")
    nc.sync.dma_start(out=bin_[:], in_=src)
    if n_cores > 1 and not _cfg.get("no_cc"):
        nc.gpsimd.collective_compute(
            "AllReduce", mybir.AluOpType.add,
            replica_groups=[list(range(n_cores))],
            ins=[bin_.opt()], outs=[bout.opt()],
        )
    else:
        nc.sync.dma_start(out=bout[:], in_=bin_[:])  # sim stand-in
    dst = sb_pool.tile([128, DM], BF16, tag="sglob")
    nc.sync.dma_start(out=dst[:], in_=bout[:])
    return dst


def _tree_n(nc, eng, tq, lo, hi, out=None):
    """In-place pairwise sum of tq rows [lo, hi) down to row lo.

    Returns the final (row-slice) AP holding the sum, or writes to out.
    """
    width = hi - lo
    while width > 1:
        half = width // 2
        odd = width - 2 * half
        nc_eng = eng
        # fold top half onto bottom half
        nc_eng.tensor_add(tq[:, lo:lo + half, :], tq[:, lo:lo + half, :],
                          tq[:, lo + half:lo + 2 * half, :])
        if odd:
            nc_eng.tensor_add(tq[:, lo, :], tq[:, lo, :],
                              tq[:, lo + 2 * half, :])
        width = half
    return tq[:, lo, :]


def _body(tc, out_ap, wt, xt, n_cores=N_CORES):
    for _rep in range(_cfg.get("reps", 1)):
        _body_once(tc, out_ap, wt, xt, n_cores)


def _body_once(tc, out_ap, wt, xt, n_cores=N_CORES):
    nc = tc.nc
    X = mybir.AxisListType.X
    ADD = mybir.AluOpType.add

    with tc.tile_pool(name="persist", bufs=1) as persist, \
         tc.tile_pool(name="dram", bufs=1, space="DRAM") as dram:
        ih = persist.tile([128, NL, DM], BF16)     # inputs_hat, 64KB/partition
        eps_t = persist.tile([128, 1], F32, tag="eps")
        nc.vector.memset(eps_t, EPS)
        # pair-lane s1 accumulators (folded at the end of the einsum)
        s1v = persist.tile([128, 2, DM], BF16, tag="s1v")
        s1g = persist.tile([128, 2, DM], BF16, tag="s1g")
        nc.vector.memset(s1v, 0.0)
        nc.gpsimd.memset(s1g, 0.0)

        # ---------------- einsum phase ----------------
        with tc.tile_pool(name="xt_pool", bufs=2) as xt_pool, \
             tc.tile_pool(name="wt_pool", bufs=2) as wt_pool, \
             tc.tile_pool(name="psum_mm", bufs=4, space="PSUM") as psum_mm:
            for nb in range(NL // NB):
                n0 = nb * NB
                xt_t = xt_pool.tile([128, 2, NB, B], BF16)
                nc.sync.dma_start(
                    out=xt_t[:],
                    in_=xt[:, n0:n0 + NB, :].rearrange(
                        "(h p) n b -> p h n b", p=128),
                )
                for pr in range(NB // 2):       # 512KB wt DMA per n-pair,
                    p0 = n0 + 2 * pr            # alternating HWDGE rings
                    wt_t = wt_pool.tile([128, 2, 2, DM], BF16,
                                        tag=f"wt_{pr % 2}")
                    dma_eng = nc.sync if pr % 2 == 0 else nc.scalar
                    dma_eng.dma_start(
                        out=wt_t[:],
                        in_=wt[p0:p0 + 2].rearrange(
                            "n (h p) m -> p n h m", p=128))
                    ps = psum_mm.tile([128, 2, DM], F32)
                    for j in range(2):
                        jx = p0 + j - n0
                        nc.tensor.matmul(ps[:, j, :], lhsT=xt_t[:, 0, jx, :],
                                         rhs=wt_t[:, j, 0, :],
                                         start=True, stop=False)
                        nc.tensor.matmul(ps[:, j, :], lhsT=xt_t[:, 1, jx, :],
                                         rhs=wt_t[:, j, 1, :],
                                         start=False, stop=True)
                    ihsl = ih[:, p0:p0 + 2, :]
                    if pr % 2 == 0:
                        nc.vector.tensor_copy(ihsl, ps)       # f32 -> bf16
                        nc.vector.tensor_add(s1v, s1v, ihsl)
                    else:
                        nc.scalar.copy(ihsl, ps)
                        nc.gpsimd.tensor_add(s1g, s1g, ihsl)

        # -------- iteration 1 (uniform c): s1 = sum_n ih / M --------
        with tc.tile_pool(name="rs0", bufs=1) as rs0:
            s1p = rs0.tile([128, 2, DM], BF16, tag="s1p")
            nc.vector.tensor_add(s1p, s1v, s1g)
            s1f = rs0.tile([128, DM], BF16, tag="s1f")
            nc.vector.tensor_add(s1f, s1p[:, 0, :], s1p[:, 1, :])
            s1_sb = rs0.tile([128, DM], BF16, tag="s1_sb")
            nc.vector.tensor_scalar_mul(s1_sb, s1f, 1.0 / M)
            s1g_t = _allreduce(tc, dram, rs0, s1_sb[:], 0, n_cores)
            oacc = persist.tile([128, DM], BF16, tag="oacc")
            o = _squash(tc, rs0, s1g_t[:], eps_t)
            nc.vector.tensor_copy(oacc, o)

        if not _cfg["routing"]:
            outf = persist.tile([128, DM], F32, tag="outf")
            nc.vector.tensor_copy(outf, oacc)
            nc.sync.dma_start(out=out_ap, in_=outf[:])
            return

        # ---------------- routing iterations 2..3 ----------------
        with tc.tile_pool(name="rp", bufs=1) as rp, \
             tc.tile_pool(name="tq_pool", bufs=2) as tq_pool, \
             tc.tile_pool(name="rsmall", bufs=1) as rsmall:
            blog = rp.tile([128, NL, M], BF16)     # routing logits
            n_iters = list(_cfg["iters"])
            GV = NH - _cfg["gn"]                   # DVE rows per half
            for it in n_iters:
                last_it = it == n_iters[-1]
                ob = oacc.unsqueeze(1)
                sh = []
                for h in range(2):
                    hsl = slice(h * NH, (h + 1) * NH)
                    # ---- logits: blog_h = sum_d oacc * ih ----
                    tq = tq_pool.tile([128, NH, DM], BF16, tag="tq")
                    for eng, lo, hi in ((nc.vector, 0, GV),
                                        (nc.gpsimd, GV, NH)):
                        rows = hi - lo
                        ksl = slice(h * NH + lo, h * NH + hi)
                        eng.tensor_mul(
                            tq[:, lo:hi, :], ih[:, ksl, :],
                            ob.broadcast_to([128, rows, DM]))
                        # tree over d (16 -> 1) in place, per-engine rows
                        tv = tq[:, lo:hi, :].rearrange(
                            "p n (d m) -> p n d m", d=D)
                        for dw in (8, 4, 2):
                            eng.tensor_add(tv[:, :, 0:dw, :],
                                           tv[:, :, 0:dw, :],
                                           tv[:, :, dw:2 * dw, :])
                        eng.tensor_add(
                            blog[:, ksl, :], tv[:, :, 0, :], tv[:, :, 1, :])
                    # ---- softmax over m (innermost free dim) ----
                    e_h = rp.tile([128, NH, M], BF16, tag=f"e{h}")
                    nc.scalar.activation(e_h, blog[:, hsl, :], EXP)
                    zt = rsmall.tile([128, NH], F32, tag=f"zt{h}")
                    nc.vector.tensor_reduce(zt, e_h, axis=X, op=ADD)
                    rz = rsmall.tile([128, NH], F32, tag=f"rz{h}")
                    nc.vector.reciprocal(rz, zt)
                    c_h = rp.tile([128, NH, M], BF16, tag=f"c{h}")
                    nc.vector.tensor_mul(
                        c_h, e_h, rz.unsqueeze(2).broadcast_to([128, NH, M]))
                    # ---- s-step: s_h = sum_n c * ih (local partial) ----
                    tq = tq_pool.tile([128, NH, DM], BF16, tag="tq")
                    gsum = []
                    for eng, lo, hi in ((nc.vector, 0, GV),
                                        (nc.gpsimd, GV, NH)):
                        rows = hi - lo
                        ksl = slice(h * NH + lo, h * NH + hi)
                        eng.tensor_mul(
                            tq[:, lo:hi, :].rearrange(
                                "p n (d m) -> p n d m", d=D),
                            ih[:, ksl, :].rearrange(
                                "p n (d m) -> p n d m", d=D),
                            c_h[:, lo:hi, :].unsqueeze(2).broadcast_to(
                                [128, rows, D, M]))
                        gsum.append(_tree_n(nc, eng, tq, lo, hi))
                    shh = rsmall.tile([128, DM], BF16, tag=f"sh{h}")
                    nc.vector.tensor_add(shh, gsum[0], gsum[1])
                    sh.append(shh)
                s_acc = rsmall.tile([128, DM], BF16, tag="s_acc")
                nc.vector.tensor_add(s_acc, sh[0], sh[1])
                sg = _allreduce(tc, dram, rsmall, s_acc[:], it - 1, n_cores)
                o = _squash(tc, rsmall, sg[:], eps_t)
                if not last_it:
                    nc.vector.tensor_add(oacc, oacc, o)

            outf = persist.tile([128, DM], F32, tag="outf")
            nc.vector.tensor_copy(outf, o)
            nc.sync.dma_start(out=out_ap, in_=outf[:])


_cache = {}


def _build(n_cores=N_CORES):
    key = ("nc", n_cores, _cfg["routing"], tuple(_cfg["iters"]),
           _cfg["reps"], _cfg.get("no_cc"), _cfg["gn"])
    if key in _cache:
        return _cache[key]
    nc = bacc.Bacc("TRN2", target_bir_lowering=False, debug=False,
                   enable_asserts=True, num_devices=n_cores)
    wt = nc.dram_tensor("wt", [NL, I, DM], BF16, kind="ExternalInput").ap()
    xt = nc.dram_tensor("xt", [I, NL, B], BF16, kind="ExternalInput").ap()
    out = nc.dram_tensor("out", [B, DM], F32, kind="ExternalOutput").ap()
    with _SteerActTables(), \
         nc.allow_low_precision(reason="bf16 routing within 2e-2 tolerance"):
        with tile.TileContext(nc) as tc:
            _body(tc, out, wt, xt, n_cores)
        nc.compile()
    _cache[key] = nc
    return nc


def make_in_maps(x, W):
    """Host-side shard prep: per-core transposed bf16 views of x and W."""
    import ml_dtypes

    mmdt = np.dtype(ml_dtypes.bfloat16)
    # WT[n, i, (d, m)] so rhs tiles [i', (d,m)] are contiguous per (n, ihalf)
    WT = np.ascontiguousarray(W.transpose(1, 3, 2, 0)).reshape(N, I, DM)
    # XT[i, n, b] so lhsT tiles [i', b] stream per n-block
    XT = np.ascontiguousarray(x.transpose(2, 1, 0))
    WT = WT.astype(mmdt)
    XT = XT.astype(mmdt)
    in_maps = []
    for c in range(N_CORES):
        sl = slice(c * NL, (c + 1) * NL)
        in_maps.append({
            "wt": WT[sl],                                   # contiguous view
            "xt": np.ascontiguousarray(XT[:, sl, :]),
        })
    return in_maps


def kernel(x, W, _trace=False):
    x = np.asarray(x, dtype=np.float32)
    W = np.asarray(W, dtype=np.float32)
    nc = _build()
    in_maps = make_in_maps(x, W)
    res = run_bass_kernel_spmd(nc, in_maps, core_ids=list(range(N_CORES)),
                               trace=_trace)
    _cache["last_result"] = res
    # ih free layout is (d, m) -> output comes back as [B, D, M]
    return res.results[0]["out"].reshape(B, D, M).transpose(0, 2, 1).copy()


# revision 24
# speedup vs baseline: 7.1332x; 7.1332x over previous
"""CapsuleLayer (dynamic routing) Trainium2 Bass kernel.

Full inputs:  x [128, 512, 256] f32, W [32, 512, 16, 256] f32
Full output:  [128, 32, 16] f32

Sharding: split the input-capsule dim N=512 across 8 cores (64 each).
Each core computes its slice of inputs_hat = einsum('bni,mndi->bmnd') in
bf16 (inputs pre-converted on host; halves the HBM traffic to ~20MB/core,
the roofline), keeps it SBUF-resident as [b=128 part, (n_loc, d, m) free]
bf16, and runs the 3 routing iterations locally (softmax over m is fully
local); the per-core partial s = sum_n c*inputs_hat is AllReduced (128KB
bf16) once per iteration.

Routing avoids DVE tensor_reduce (always 1 elem/cycle) in favor of
pairwise in-place add-trees on TensorTensor ops, which run at 2x in bf16
with packed innermost access.  Work is split by n-rows between DVE and
GPSIMD, each engine cascading its own rows through mul + tree so there
are no cross-engine deps until the softmax.  Iteration i's logits are
recomputed from o_sum = sum_t o_t (b-updates are linear in o), so no
logit state is carried.  Iteration 1 (uniform c) reduces to
s1 = sum_n ih / M, accumulated with bf16 adds hidden under the einsum
DMA.  Squash uses exp(-0.5*ln(u+eps)) for rsqrt so the whole kernel
lives in one ACT function set (no table reloads).
"""

import sys

sys.path.insert(0, "/opt/trn_rl_repo")

import numpy as np

import concourse.bacc as bacc
import concourse.mybir as mybir
import concourse.tile as tile
from concourse.bass_utils import run_bass_kernel_spmd

N_CORES = 8
B, N, I = 128, 512, 256
M, D = 32, 16
DM = D * M                 # ih free layout is (d, m): m innermost
NL = N // N_CORES          # 64 local input capsules per core
NH = NL // 2               # routing processes n in two halves of 32
EPS = 1e-7
F32 = mybir.dt.float32
BF16 = mybir.dt.bfloat16

NB = 8                     # n-block size per xt DMA / wt DMA pair

# debug/profiling knobs (defaults = full kernel)
# gn: GPSIMD n-rows per 32-row half (DVE gets 32-gn)
_cfg = {"routing": True, "iters": (2, 3), "reps": 1, "no_cc": False, "gn": 6}

LN = mybir.ActivationFunctionType.Ln
EXP = mybir.ActivationFunctionType.Exp


class _SteerActTables:
    """Compile-scoped: hide exp/ln from every act-func set except
    natural_log_exp_and_others so the table selector settles on the one
    set that covers all our activations (Copy/Exp/Ln) — a single table
    load for the whole kernel instead of per-iteration thrash.  Set ids
    keep their act_info.json indices, so the chosen table is still valid;
    the original resolver is restored on exit."""

    def __enter__(self):
        self._orig = bacc.get_activation_tables

        def steered(arch):
            tabs = self._orig(arch)
            keep = "natural_log_exp_and_others"
            out = {}
            for name, funcs in tabs.items():
                if name != keep:
                    funcs = funcs - {mybir.ActivationFunctionType.Exp,
                                     mybir.ActivationFunctionType.Ln}
                out[name] = funcs
            return out

        bacc.get_activation_tables = steered
        return self

    def __exit__(self, *a):
        bacc.get_activation_tables = self._orig
        return False


def _rsqrt(nc, pool, u, eps_t, tag):
    """exp(-0.5*ln(u+eps)) on ACT: both funcs in one act set."""
    lg = pool.tile([128, M], F32, tag="lg" + tag)
    nc.scalar.activation(lg, u, LN, bias=eps_t[:, 0:1])
    ri = pool.tile([128, M], F32, tag="ri" + tag)
    nc.scalar.activation(ri, lg, EXP, scale=-0.5)
    return ri


def _squash(tc, pool, s_src, eps_t, tag="", out_dtype=BF16):
    """o = squash(s) over d; s [128, (d, m)] f32 SBUF."""
    nc = tc.nc
    sq = pool.tile([128, DM], F32, tag="sq" + tag)
    nc.vector.tensor_mul(sq, s_src, s_src)
    s2 = pool.tile([128, M], F32, tag="s2" + tag)
    nc.vector.tensor_reduce(
        s2, sq.rearrange("p (d m) -> p m d", d=D),
        axis=mybir.AxisListType.X, op=mybir.AluOpType.add,
    )
    ri = _rsqrt(nc, pool, s2, eps_t, tag)   # 1/sqrt(s2+eps)
    one_p = pool.tile([128, M], F32, tag="one_p" + tag)
    nc.vector.tensor_scalar_add(one_p, s2, 1.0)
    rec = pool.tile([128, M], F32, tag="rec" + tag)
    nc.vector.reciprocal(rec, one_p)
    pr = pool.tile([128, M], F32, tag="pr" + tag)
    nc.vector.tensor_mul(pr, s2, rec)
    scl = pool.tile([128, M], F32, tag="scl" + tag)
    nc.vector.tensor_mul(scl, pr, ri)       # scale = s2/(1+s2)/sqrt(s2+eps)
    o = pool.tile([128, DM], out_dtype, tag="o" + tag)
    nc.vector.tensor_mul(
        o.rearrange("p (d m) -> p d m", d=D),
        s_src.rearrange("p (d m) -> p d m", d=D),
        scl.unsqueeze(1).broadcast_to([128, D, M]),
    )
    return o


def _allreduce(tc, dram_pool, sb_pool, src, idx, n_cores=N_CORES):
    """AllReduce [128, DM] f32 across the cores. Returns SBUF tile."""
    nc = tc.nc
    bin_ = dram_pool.tile([128, DM], F32, tag=f"arin{idx}")
    bout = dram_pool.tile([128, DM], F32, tag=f"arout{idx}")
    nc.sync.dma_start(out=bin_[:], in_=src)
    if n_cores > 1 and not _cfg.get("no_cc"):
        nc.gpsimd.collective_compute(
            "AllReduce", mybir.AluOpType.add,
            replica_groups=[list(range(n_cores))],
            ins=[bin_.opt()], outs=[bout.opt()],
        )
    else:
        nc.sync.dma_start(out=bout[:], in_=bin_[:])  # sim stand-in
    dst = sb_pool.tile([128, DM], F32, tag="sglob")
    nc.sync.dma_start(out=dst[:], in_=bout[:])
    return dst


def _tree_n(nc, eng, tq, lo, hi, out=None):
    """In-place pairwise sum of tq rows [lo, hi) down to row lo.

    Returns the final (row-slice) AP holding the sum, or writes to out.
    """
    width = hi - lo
    while width > 1:
        half = width // 2
        odd = width - 2 * half
        nc_eng = eng
        # fold top half onto bottom half
        nc_eng.tensor_add(tq[:, lo:lo + half, :], tq[:, lo:lo + half, :],
                          tq[:, lo + half:lo + 2 * half, :])
        if odd:
            nc_eng.tensor_add(tq[:, lo, :], tq[:, lo, :],
                              tq[:, lo + 2 * half, :])
        width = half
    return tq[:, lo, :]


def _body(tc, out_ap, wt, xt, n_cores=N_CORES):
    for _rep in range(_cfg.get("reps", 1)):
        _body_once(tc, out_ap, wt, xt, n_cores)


def _body_once(tc, out_ap, wt, xt, n_cores=N_CORES):
    nc = tc.nc
    X = mybir.AxisListType.X
    ADD = mybir.AluOpType.add

    with tc.tile_pool(name="persist", bufs=1) as persist, \
         tc.tile_pool(name="dram", bufs=1, space="DRAM") as dram:
        ih = persist.tile([128, NL, DM], BF16)     # inputs_hat, 64KB/partition
        eps_t = persist.tile([128, 1], F32, tag="eps")
        nc.vector.memset(eps_t, EPS)
        # pair-lane f32 s1 accumulators (folded at the end of the einsum);
        # f32 keeps the 64-add chain off the error budget
        s1v = persist.tile([128, 2, DM], F32, tag="s1v")
        s1g = persist.tile([128, 2, DM], F32, tag="s1g")
        nc.vector.memset(s1v, 0.0)
        nc.gpsimd.memset(s1g, 0.0)

        # ---------------- einsum phase ----------------
        with tc.tile_pool(name="xt_pool", bufs=2) as xt_pool, \
             tc.tile_pool(name="wt_pool", bufs=2) as wt_pool, \
             tc.tile_pool(name="psum_mm", bufs=4, space="PSUM") as psum_mm:
            for nb in range(NL // NB):
                n0 = nb * NB
                xt_t = xt_pool.tile([128, 2, NB, B], BF16)
                nc.sync.dma_start(
                    out=xt_t[:],
                    in_=xt[:, n0:n0 + NB, :].rearrange(
                        "(h p) n b -> p h n b", p=128),
                )
                for pr in range(NB // 2):       # 512KB wt DMA per n-pair,
                    p0 = n0 + 2 * pr            # alternating HWDGE rings
                    wt_t = wt_pool.tile([128, 2, 2, DM], BF16,
                                        tag=f"wt_{pr % 2}")
                    dma_eng = nc.sync if pr % 2 == 0 else nc.scalar
                    dma_eng.dma_start(
                        out=wt_t[:],
                        in_=wt[p0:p0 + 2].rearrange(
                            "n (h p) m -> p n h m", p=128))
                    ps = psum_mm.tile([128, 2, DM], F32)
                    for j in range(2):
                        jx = p0 + j - n0
                        nc.tensor.matmul(ps[:, j, :], lhsT=xt_t[:, 0, jx, :],
                                         rhs=wt_t[:, j, 0, :],
                                         start=True, stop=False)
                        nc.tensor.matmul(ps[:, j, :], lhsT=xt_t[:, 1, jx, :],
                                         rhs=wt_t[:, j, 1, :],
                                         start=False, stop=True)
                    ihsl = ih[:, p0:p0 + 2, :]
                    if pr % 2 == 0:
                        nc.vector.tensor_copy(ihsl, ps)       # f32 -> bf16
                        nc.vector.tensor_add(s1v, s1v, ihsl)
                    else:
                        nc.scalar.copy(ihsl, ps)
                        nc.gpsimd.tensor_add(s1g, s1g, ihsl)

        # -------- iteration 1 (uniform c): s1 = sum_n ih / M --------
        with tc.tile_pool(name="rs0", bufs=1) as rs0:
            s1p = rs0.tile([128, 2, DM], F32, tag="s1p")
            nc.vector.tensor_add(s1p, s1v, s1g)
            s1f = rs0.tile([128, DM], F32, tag="s1f")
            nc.vector.tensor_add(s1f, s1p[:, 0, :], s1p[:, 1, :])
            s1_sb = rs0.tile([128, DM], F32, tag="s1_sb")
            nc.vector.tensor_scalar_mul(s1_sb, s1f, 1.0 / M)
            s1g_t = _allreduce(tc, dram, rs0, s1_sb[:], 0, n_cores)
            oacc = persist.tile([128, DM], BF16, tag="oacc")
            o = _squash(tc, rs0, s1g_t[:], eps_t)
            nc.vector.tensor_copy(oacc, o)

        if not _cfg["routing"]:
            outf = persist.tile([128, DM], F32, tag="outf")
            nc.vector.tensor_copy(outf, oacc)
            nc.sync.dma_start(out=out_ap, in_=outf[:])
            return

        # ---------------- routing iterations 2..3 ----------------
        with tc.tile_pool(name="rp", bufs=1) as rp, \
             tc.tile_pool(name="tq_pool", bufs=2) as tq_pool, \
             tc.tile_pool(name="rsmall", bufs=1) as rsmall:
            blog = rp.tile([128, NL, M], BF16)     # routing logits
            n_iters = list(_cfg["iters"])
            GV = NH - _cfg["gn"]                   # DVE rows per half
            for it in n_iters:
                last_it = it == n_iters[-1]
                ob = oacc.unsqueeze(1)
                sh = []
                for h in range(2):
                    hsl = slice(h * NH, (h + 1) * NH)
                    # ---- logits: blog_h = sum_d oacc * ih ----
                    tq = tq_pool.tile([128, NH, DM], BF16, tag="tq")
                    for eng, lo, hi in ((nc.vector, 0, GV),
                                        (nc.gpsimd, GV, NH)):
                        rows = hi - lo
                        ksl = slice(h * NH + lo, h * NH + hi)
                        eng.tensor_mul(
                            tq[:, lo:hi, :], ih[:, ksl, :],
                            ob.broadcast_to([128, rows, DM]))
                        # tree over d (16 -> 1) in place, per-engine rows
                        tv = tq[:, lo:hi, :].rearrange(
                            "p n (d m) -> p n d m", d=D)
                        for dw in (8, 4, 2):
                            eng.tensor_add(tv[:, :, 0:dw, :],
                                           tv[:, :, 0:dw, :],
                                           tv[:, :, dw:2 * dw, :])
                        eng.tensor_add(
                            blog[:, ksl, :], tv[:, :, 0, :], tv[:, :, 1, :])
                    # ---- softmax over m (innermost free dim) ----
                    e_h = rp.tile([128, NH, M], BF16, tag=f"e{h}")
                    nc.scalar.activation(e_h, blog[:, hsl, :], EXP)
                    zt = rsmall.tile([128, NH], F32, tag=f"zt{h}")
                    nc.vector.tensor_reduce(zt, e_h, axis=X, op=ADD)
                    rz = rsmall.tile([128, NH], F32, tag=f"rz{h}")
                    nc.vector.reciprocal(rz, zt)
                    c_h = rp.tile([128, NH, M], BF16, tag=f"c{h}")
                    nc.vector.tensor_mul(
                        c_h, e_h, rz.unsqueeze(2).broadcast_to([128, NH, M]))
                    # ---- s-step: s_h = sum_n c * ih (local partial) ----
                    tq = tq_pool.tile([128, NH, DM], BF16, tag="tq")
                    gsum = []
                    for eng, lo, hi in ((nc.vector, 0, GV),
                                        (nc.gpsimd, GV, NH)):
                        rows = hi - lo
                        ksl = slice(h * NH + lo, h * NH + hi)
                        eng.tensor_mul(
                            tq[:, lo:hi, :].rearrange(
                                "p n (d m) -> p n d m", d=D),
                            ih[:, ksl, :].rearrange(
                                "p n (d m) -> p n d m", d=D),
                            c_h[:, lo:hi, :].unsqueeze(2).broadcast_to(
                                [128, rows, D, M]))
                        gsum.append(_tree_n(nc, eng, tq, lo, hi))
                    shh = rsmall.tile([128, DM], F32, tag=f"sh{h}")
                    nc.vector.tensor_add(shh, gsum[0], gsum[1])
                    sh.append(shh)
                s_acc = rsmall.tile([128, DM], F32, tag="s_acc")
                nc.vector.tensor_add(s_acc, sh[0], sh[1])
                sg = _allreduce(tc, dram, rsmall, s_acc[:], it - 1, n_cores)
                o = _squash(tc, rsmall, sg[:], eps_t,
                            out_dtype=F32 if last_it else BF16)
                if not last_it:
                    nc.vector.tensor_add(oacc, oacc, o)

            nc.sync.dma_start(out=out_ap, in_=o[:])


_cache = {}


def _build(n_cores=N_CORES):
    key = ("nc", n_cores, _cfg["routing"], tuple(_cfg["iters"]),
           _cfg["reps"], _cfg.get("no_cc"), _cfg["gn"])
    if key in _cache:
        return _cache[key]
    nc = bacc.Bacc("TRN2", target_bir_lowering=False, debug=False,
                   enable_asserts=True, num_devices=n_cores)
    wt = nc.dram_tensor("wt", [NL, I, DM], BF16, kind="ExternalInput").ap()
    xt = nc.dram_tensor("xt", [I, NL, B], BF16, kind="ExternalInput").ap()
    out = nc.dram_tensor("out", [B, DM], F32, kind="ExternalOutput").ap()
    with _SteerActTables(), \
         nc.allow_low_precision(reason="bf16 routing within 2e-2 tolerance"):
        with tile.TileContext(nc) as tc:
            _body(tc, out, wt, xt, n_cores)
        nc.compile()
    _cache[key] = nc
    return nc


def make_in_maps(x, W):
    """Host-side shard prep: per-core transposed bf16 views of x and W."""
    import ml_dtypes

    mmdt = np.dtype(ml_dtypes.bfloat16)
    # WT[n, i, (d, m)] so rhs tiles [i', (d,m)] are contiguous per (n, ihalf)
    WT = np.ascontiguousarray(W.transpose(1, 3, 2, 0)).reshape(N, I, DM)
    # XT[i, n, b] so lhsT tiles [i', b] stream per n-block
    XT = np.ascontiguousarray(x.transpose(2, 1, 0))
    WT = WT.astype(mmdt)
    XT = XT.astype(mmdt)
    in_maps = []
    for c in range(N_CORES):
        sl = slice(c * NL, (c + 1) * NL)
        in_maps.append({
            "wt": WT[sl],                                   # contiguous view
            "xt": np.ascontiguousarray(XT[:, sl, :]),
        })
    return in_maps


def kernel(x, W, _trace=False):
    x = np.asarray(x, dtype=np.float32)
    W = np.asarray(W, dtype=np.float32)
    nc = _build()
    in_maps = make_in_maps(x, W)
    res = run_bass_kernel_spmd(nc, in_maps, core_ids=list(range(N_CORES)),
                               trace=_trace)
    _cache["last_result"] = res
    # ih free layout is (d, m) -> output comes back as [B, D, M]
    return res.results[0]["out"].reshape(B, D, M).transpose(0, 2, 1).copy()
